# revision 11
# baseline (speedup 1.0000x reference)
"""Grouped-Query Attention on 8 Trainium2 NeuronCores.

Sharding: core c handles (batch b = c//4, query-head group g = c%4).
Each core computes its group's Q projection (256 cols of W_Q), the
group-shared K/V projections, 4 heads of attention over the full
sequence, and a partial output projection against the group's 256 rows
of W_O. The host sums the 4 group partials per batch (the "all-reduce")
and adds b_O.

On-core dataflow (all matmuls bf16 operands, fp32 PSUM accumulate):
  xbf  = bf16(x)                       DVE cast
  xT   = transpose(xbf)                PE transpose (bf16, 1 cyc/row)
  [K|V]^T = [wk|wv]^T x  per k-chunk   one packed matmul; K rows 0:64,
                                       V^T rows 64:128 of PSUM
  V    = transpose(V^T)                PE, 16 small transposes
  QT   = W_Q^T x  (q-dim on partitions), + b_Q on evac
  S^T  = KT_h^T @ QT_h   per head, [t, s] layout; head pairs stream
         concurrently on PE row-tiles (0,0)/(64,0)
  P^T  = exp(S^T / 8)    ScalarE, PSUM -> SBUF bf16
  ctx  = P^T_chunk^T @ [V | 1]   accumulated PER TOKEN CHUNK, 2 chunks
         behind the exp stream, so P^T lives in a 3-slot ring and the
         Scalar engine (the bottleneck) never waits on context work
  ctx /= denom; transpose -> ctxT; out = ctxT^T @ W_O (partial, fp32);
  ctxT/out for super-block sb run inside sb+1's score stream.

b_V and b_O are applied on the host: b_V adds exactly
(tile(b_V) @ W_O_g) to every output row (softmax weights sum to 1).
"""

import numpy as np

S = 2048
DM = 1024
G = 4
H = 4  # heads per group
DK = 64
GQ = 256  # query width per group
B = 2
NK = DM // 128  # 8 contraction chunks
NT = S // 128  # 16 token chunks
SBLK = 512
NSB = S // SBLK  # 4 query super-blocks
PTS = 12  # P^T ring slots (token chunks in flight)

_CACHED = {}


def _split_sync_waits(nc, drain_max=1, other_max=1):
    """This walrus build has a single sync-wait slot on CTRL-class
    instructions (Drain/NoOp); Tile's exit drain collects 3+. Move the
    excess onto preceding single-wait NOPs on the same engine."""
    import concourse.mybir as mybir
    import bass_rust

    n_split = 0
    for f in nc.m.functions:
        for bb in f.blocks:
            out = []
            changed = False
            for inst in bb.instructions:
                si = getattr(inst, "sync_info", None)
                limit = drain_max if type(inst).__name__ in ("InstDrain", "InstNoOp") else other_max
                if si is not None and len(si.on_wait) > limit:
                    waits = list(si.on_wait)
                    keep = waits[-limit:] if limit else []
                    head = waits[: len(waits) - limit]
                    for w in head:
                        out.append(
                            mybir.InstNoOp(
                                name=f"{inst.name}-wsp{n_split}",
                                engine=inst.engine,
                                sync_info=mybir.SyncInfo(on_wait=[w], on_update=[]),
                                bass_nofuse=True,
                            )
                        )
                        n_split += 1
                    inst.sync_info = bass_rust.SyncInfo(on_wait=keep, on_update=si.on_update)
                    changed = True
                out.append(inst)
            if changed:
                bb.instructions = out
    return n_split


def _build_nc(iters=1):
    import concourse.bass as bass
    import concourse.mybir as mybir
    import concourse.tile as tile
    from concourse import masks

    F32 = mybir.dt.float32
    BF = mybir.dt.bfloat16

    nc = bass.Bass("TRN2", target_bir_lowering=False, debug=False, num_devices=8)
    x = nc.dram_tensor("x", [S, DM], F32, kind="ExternalInput")
    wq = nc.dram_tensor("wq", [DM, GQ], F32, kind="ExternalInput")
    wk = nc.dram_tensor("wk", [DM, DK], F32, kind="ExternalInput")
    wv = nc.dram_tensor("wv", [DM, DK], F32, kind="ExternalInput")
    wo = nc.dram_tensor("wo", [GQ, DM], F32, kind="ExternalInput")
    bq = nc.dram_tensor("bq", [GQ], F32, kind="ExternalInput")
    bk = nc.dram_tensor("bk", [DK], F32, kind="ExternalInput")
    out = nc.dram_tensor("out", [S, DM], F32, kind="ExternalOutput")

    with tile.TileContext(nc) as tc:
        with (
            tc.tile_pool(name="const", bufs=1) as cpool,
            tc.tile_pool(name="wstg", bufs=3) as wstg,
            tc.tile_pool(name="wts", bufs=1) as wts,
            tc.tile_pool(name="xin", bufs=2) as xin,
            tc.tile_pool(name="acts", bufs=1) as acts,
            tc.tile_pool(name="outp", bufs=2) as outp,
            tc.tile_pool(name="ps_sc", bufs=2, space="PSUM") as ps_sc,
            tc.tile_pool(name="ps_ctx", bufs=4, space="PSUM") as ps_ctx,
        ):
            def _pipeline():
                # ---- constants ----
                ident_b = cpool.tile([128, 128], BF)
                masks.make_identity(nc, ident_b[:])
                # ident_lo[64+i, j] = delta(i, j): identity content on
                # partitions 64:128, for transposing stationaries that
                # live on the lower half.
                ident_lo = cpool.tile([128, 64], BF)
                nc.sync.dma_start(ident_lo[64:128, :], ident_b[:64, :64])
                bq_t = cpool.tile([128, 2], F32)
                for m in range(2):
                    nc.sync.dma_start(bq_t[:, m : m + 1], bq[m * 128 : (m + 1) * 128])
                bk_t = cpool.tile([64, 1], F32)
                nc.sync.dma_start(bk_t[:], bk[:])

                # ---- weights: stage fp32 (batched 3D-AP DMAs), cast bf16 ----
                wq_bf = wts.tile([128, NK * GQ], BF)  # chunk k at cols [k*GQ,)
                kv_bf = wts.tile([128, NK * 128], BF)  # chunk k: [wk_k | wv_k]
                wo_bf = wts.tile([128, 2 * DM], BF)  # chunk cj at cols [cj*DM,)

                stg2 = wstg.tile([128, NK * GQ], F32, tag="stg")
                kv_view = stg2[:, : NK * 128].rearrange("p (k t q) -> p k t q", t=2, q=DK)
                nc.sync.dma_start(
                    kv_view[:, :, 0, :], wk[:].rearrange("(k p) q -> p k q", p=128)
                )
                nc.sync.dma_start(
                    kv_view[:, :, 1, :], wv[:].rearrange("(k p) q -> p k q", p=128)
                )
                nc.vector.tensor_copy(kv_bf[:], stg2[:, : NK * 128])

                stg = wstg.tile([128, NK * GQ], F32, tag="stg")
                nc.sync.dma_start(
                    stg[:].rearrange("p (k q) -> p k q", q=GQ),
                    wq[:].rearrange("(k p) q -> p k q", p=128),
                )
                nc.vector.tensor_copy(wq_bf[:], stg[:])

                stg3 = wstg.tile([128, NK * GQ], F32, tag="stg")
                nc.sync.dma_start(
                    stg3[:].rearrange("p (c n) -> p c n", n=DM),
                    wo[:].rearrange("(c p) n -> p c n", p=128),
                )
                nc.vector.tensor_copy(wo_bf[:], stg3[:])

                # ---- activations ----
                xT = acts.tile([128, NK * S], BF)  # xT[:, k*S + s]
                KT = acts.tile([128, S], BF)  # rows 64-127 duplicate 0-63
                VT = acts.tile([128, S], BF)  # V^T on rows 64:128
                Vb = acts.tile([128, NT * (DK + 1)], BF)  # [V | ones] per chunk
                QT = acts.tile([128, 2 * S], BF)  # m-tile m: heads 2m, 2m+1
                PT = acts.tile([128, PTS * H * SBLK], BF)  # ring, slot g%PTS
                ctx_ab = [
                    acts.tile([128, 4 * GQ], BF, name=f"ctx{i}") for i in range(2)
                ]
                ctxT_ab = [
                    acts.tile([128, 2 * SBLK], BF, name=f"ctxT{i}") for i in range(2)
                ]
                ctxA = acts.tile([128, 4 * 4 * (DK + 1)], F32)  # pass-A partials
                ctxs = acts.tile([128, 4 * 4 * (DK + 1)], F32)  # A+B sums

                nc.vector.memset(
                    Vb[:].rearrange("p (t c) -> p t c", c=65)[:, :, DK], 1.0
                )

                xfs = {}

                def load_sg(sg):
                    xf = xin.tile([128, 4 * DM], F32, tag="xf")
                    nc.sync.dma_start(
                        xf[:].rearrange("p (c d) -> p c d", d=DM),
                        x[sg * SBLK : (sg + 1) * SBLK, :].rearrange(
                            "(c p) d -> p c d", p=128
                        ),
                    )
                    xfs[sg] = xf

                def sg_tr(sg):
                    """cast + transpose for one token super-block."""
                    if sg + 1 < NSB:
                        load_sg(sg + 1)
                    xbf = xin.tile([128, 4 * DM], BF, tag="xbf", bufs=1)
                    nc.vector.tensor_copy(xbf[:], xfs.pop(sg)[:])
                    for k in range(NK):
                        ps = ps_sc.tile([128, 2 * SBLK], BF, tag="sc", name="pstr")
                        for i in range(4):
                            nc.tensor.transpose(
                                ps[:, i * 128 : (i + 1) * 128],
                                xbf[:, i * DM + k * 128 : i * DM + (k + 1) * 128],
                                ident_b[:],
                            )
                        nc.vector.tensor_copy(
                            xT[:, k * S + sg * SBLK : k * S + (sg + 1) * SBLK],
                            ps[:, :SBLK],
                        )

                def sg_kv(sg):
                    """packed K/V proj + V chunks + KT dup."""
                    ps2 = ps_sc.tile([128, 2 * SBLK], F32, tag="sc", name="pskv")
                    for k in range(NK):
                        nc.tensor.matmul(
                            ps2[:, :SBLK],
                            kv_bf[:, k * 128 : (k + 1) * 128],
                            xT[:, k * S + sg * SBLK : k * S + (sg + 1) * SBLK],
                            start=(k == 0),
                            stop=(k == NK - 1),
                        )
                    nc.vector.tensor_scalar_add(
                        KT[:64, sg * SBLK : (sg + 1) * SBLK], ps2[:64, :SBLK], bk_t[:]
                    )
                    nc.vector.tensor_copy(
                        VT[64:128, sg * SBLK : (sg + 1) * SBLK], ps2[64:128, :SBLK]
                    )
                    nc.sync.dma_start(
                        KT[64:128, sg * SBLK : (sg + 1) * SBLK],
                        KT[:64, sg * SBLK : (sg + 1) * SBLK],
                    )
                    for t in range(sg * 4, sg * 4 + 4):
                        vps = ps_sc.tile([128, 2 * SBLK], BF, tag="sc", name="vtr")
                        nc.tensor.transpose(
                            vps[:, :DK],
                            VT[64:128, t * 128 : (t + 1) * 128],
                            ident_lo[64:128, :],
                        )
                        nc.vector.tensor_copy(Vb[:, t * 65 : t * 65 + DK], vps[:, :DK])

                def sg_block(sg):
                    sg_tr(sg)
                    sg_kv(sg)

                def q_block(m, sb):
                    ps = ps_sc.tile([128, 2 * SBLK], F32, tag="sc", name="psq")
                    for k in range(NK):
                        nc.tensor.matmul(
                            ps[:, :SBLK],
                            wq_bf[:, k * GQ + m * 128 : k * GQ + (m + 1) * 128],
                            xT[:, k * S + sb * SBLK : k * S + (sb + 1) * SBLK],
                            start=(k == 0),
                            stop=(k == NK - 1),
                        )
                    nc.vector.tensor_scalar_add(
                        QT[:, m * S + sb * SBLK : m * S + (sb + 1) * SBLK],
                        ps[:, :SBLK],
                        bq_t[:, m : m + 1],
                    )

                def scores_t(sb, t):
                    """score pair matmuls + exp for one token chunk into the
                    P^T ring slot (sb*NT + t) % PTS."""
                    slot = ((sb * NT + t) % PTS) * H * SBLK
                    for p in range(2):
                        sc = ps_sc.tile([128, 2 * SBLK], F32, tag="sc", name="scs")
                        for hl in range(2):
                            nc.tensor.matmul(
                                sc[:, hl * SBLK : (hl + 1) * SBLK],
                                KT[hl * 64 : (hl + 1) * 64, t * 128 : (t + 1) * 128],
                                QT[hl * 64 : (hl + 1) * 64,
                                   p * S + sb * SBLK : p * S + (sb + 1) * SBLK],
                            )
                        nc.scalar.activation(
                            PT[:, slot + 2 * p * SBLK : slot + (2 * p + 2) * SBLK],
                            sc[:],
                            mybir.ActivationFunctionType.Exp,
                            scale=0.125,
                        )

                # PSUM accumulation groups must be contiguous per tile, so
                # each (head, sc) context reduction runs as two sequential
                # half-chains (t 0-7 and 8-15); pass A parks in SBUF fp32 and
                # a DVE add combines the passes.
                cps_by_sb = {}

                def ctx_burst(sb, h, t0, t1):
                    if sb not in cps_by_sb:
                        cps_by_sb[sb] = [
                            ps_ctx.tile([128, 4 * (DK + 1)], F32, tag="c", name=f"cps{i}")
                            for i in range(H)
                        ]
                    cps = cps_by_sb[sb][h]
                    for sc_i in range(4):
                        for t in range(t0, t1 + 1):
                            slot = ((sb * NT + t) % PTS) * H * SBLK
                            nc.tensor.matmul(
                                cps[:, sc_i * 65 : sc_i * 65 + 65],
                                PT[:, slot + h * SBLK + sc_i * 128 : slot + h * SBLK + (sc_i + 1) * 128],
                                Vb[:, t * 65 : (t + 1) * 65],
                                start=(t == t0),
                                stop=(t == t1),
                            )
                    if t0 == 0:  # pass A: park partials in SBUF
                        nc.vector.tensor_copy(
                            ctxA[:, h * 260 : (h + 1) * 260], cps[:]
                        )

                def ctx_addevac(sb):
                    ctx_sb = ctx_ab[sb % 2]
                    cps = cps_by_sb.pop(sb)
                    for h in range(H):
                        nc.vector.tensor_add(
                            ctxs[:, h * 260 : (h + 1) * 260],
                            cps[h][:],
                            ctxA[:, h * 260 : (h + 1) * 260],
                        )
                    for h in range(H):
                        rc = xin.tile([128, 4], F32, tag="rc")
                        nc.vector.reciprocal(
                            rc[:],
                            ctxs[:, h * 260 : (h + 1) * 260].rearrange(
                                "p (sc c) -> p sc c", c=65
                            )[:, :, DK],
                        )
                        for sc_i in range(4):
                            nc.vector.tensor_scalar_mul(
                                ctx_sb[:, sc_i * GQ + h * DK : sc_i * GQ + (h + 1) * DK],
                                ctxs[:, h * 260 + sc_i * 65 : h * 260 + sc_i * 65 + DK],
                                rc[:, sc_i : sc_i + 1],
                            )

                def ctxT_block(sb):
                    ctx_sb, ctxT_sb = ctx_ab[sb % 2], ctxT_ab[sb % 2]
                    for cj in range(2):
                        ps = ps_sc.tile([128, 2 * SBLK], BF, tag="sc", name="pst")
                        for sc_i in range(4):
                            nc.tensor.transpose(
                                ps[:, sc_i * 128 : (sc_i + 1) * 128],
                                ctx_sb[:, sc_i * GQ + cj * 128 : sc_i * GQ + (cj + 1) * 128],
                                ident_b[:],
                            )
                        nc.vector.tensor_copy(
                            ctxT_sb[:, cj * SBLK : (cj + 1) * SBLK], ps[:, :SBLK]
                        )

                def out_half(sb, half):
                    ctxT_sb = ctxT_ab[sb % 2]
                    ot = outp.tile([128, 2 * DM], F32, tag="ot")
                    for ci in range(2):
                        sc_i = half * 2 + ci
                        ps = ps_sc.tile([128, 2 * SBLK], F32, tag="sc", name="pso")
                        for nb in range(2):
                            for cj in range(2):
                                nc.tensor.matmul(
                                    ps[:, nb * SBLK : (nb + 1) * SBLK],
                                    ctxT_sb[:, cj * SBLK + sc_i * 128 : cj * SBLK + (sc_i + 1) * 128],
                                    wo_bf[:, cj * DM + nb * SBLK : cj * DM + (nb + 1) * SBLK],
                                    start=(cj == 0),
                                    stop=(cj == 1),
                                )
                        nc.vector.tensor_copy(ot[:, ci * DM : (ci + 1) * DM], ps[:, :DM])
                    row = sb * SBLK + half * 256
                    nc.sync.dma_start(
                        out[row : row + 256, :].rearrange("(c p) d -> p c d", p=128),
                        ot[:].rearrange("p (c d) -> p c d", d=DM),
                    )

                def stream_sb(sb, prework):
                    """One super-block's score/exp stream; prework[t] emits
                    extra PE work after scores_t(sb, t) to fill the
                    ACT-paced slack. Pass-A ctx bursts (one head per slot)
                    run at t=8..11 once their P^T chunks exist."""
                    for t in range(NT):
                        scores_t(sb, t)
                        if 8 <= t <= 11:
                            ctx_burst(sb, t - 8, 0, 7)
                        for fn in prework.get(t, ()):
                            fn()

                # ---- software-pipelined schedule ----
                load_sg(0)
                sg_block(0)
                q_block(0, 0)
                q_block(1, 0)
                stream_sb(
                    0,
                    {
                        3: [lambda: sg_block(1)],
                        7: [lambda: sg_block(2)],
                        11: [lambda: sg_block(3)],
                        14: [lambda: q_block(0, 1)],
                        15: [lambda: q_block(1, 1)],
                    },
                )
                for sb in range(1, NSB):
                    prev = sb - 1
                    prework = {
                        0: [lambda: ctx_burst(prev, 0, 8, NT - 1)],
                        1: [lambda: ctx_burst(prev, 1, 8, NT - 1)],
                        2: [lambda: ctx_burst(prev, 2, 8, NT - 1)],
                        3: [lambda: ctx_burst(prev, 3, 8, NT - 1)],
                        4: [lambda: ctx_addevac(prev)],
                        5: [lambda: ctxT_block(prev)],
                        6: [lambda: out_half(prev, 0)],
                        7: [lambda: out_half(prev, 1)],
                    }
                    if sb + 1 < NSB:
                        prework[14] = [lambda: q_block(0, sb + 1)]
                        prework[15] = [lambda: q_block(1, sb + 1)]
                    stream_sb(sb, prework)

                last = NSB - 1
                for h in range(H):
                    ctx_burst(last, h, 8, NT - 1)
                ctx_addevac(last)
                ctxT_block(last)
                out_half(last, 0)
                out_half(last, 1)

            if iters == 1:
                _pipeline()
            else:
                with tc.For_i(0, iters):
                    _pipeline()

    _split_sync_waits(nc)
    return nc


def kernel(x, W_Q, b_Q, W_K, b_K, W_V, b_V, W_O, b_O):
    from concourse.bass_utils import run_bass_kernel_spmd

    x = np.asarray(x, np.float32)
    W_Q, b_Q = np.asarray(W_Q, np.float32), np.asarray(b_Q, np.float32)
    W_K, b_K = np.asarray(W_K, np.float32), np.asarray(b_K, np.float32)
    W_V, b_V = np.asarray(W_V, np.float32), np.asarray(b_V, np.float32)
    W_O, b_O = np.asarray(W_O, np.float32), np.asarray(b_O, np.float32)

    if "nc" not in _CACHED:
        _CACHED["nc"] = _build_nc()
    nc = _CACHED["nc"]

    in_maps = []
    for c in range(8):
        b, g = divmod(c, 4)
        in_maps.append(
            {
                "x": np.ascontiguousarray(x[b]),
                "wq": np.ascontiguousarray(W_Q[:, g * GQ : (g + 1) * GQ]),
                "wk": np.ascontiguousarray(W_K[g]),
                "wv": np.ascontiguousarray(W_V[g]),
                "wo": np.ascontiguousarray(W_O[g * GQ : (g + 1) * GQ, :]),
                "bq": np.ascontiguousarray(b_Q[g * GQ : (g + 1) * GQ]),
                "bk": np.ascontiguousarray(b_K[g]),
            }
        )
    res = run_bass_kernel_spmd(nc, in_maps, list(range(8)))

    out = np.zeros((B, S, DM), np.float32)
    for c in range(8):
        b, g = divmod(c, 4)
        out[b] += res.results[c]["out"]
    # host-side bias terms: b_O, plus b_V's exact contribution
    # (softmax rows sum to 1 -> ctx bias = tile(b_V[g]) per head)
    bv_full = np.concatenate([np.tile(b_V[g], H) for g in range(G)])  # [1024]
    out += (b_O + bv_full @ W_O)[None, None, :]
    return out


# revision 26
# speedup vs baseline: 1.4487x; 1.4487x over previous
"""Grouped-Query Attention on 8 Trainium2 NeuronCores.

Sharding: core c handles (batch b = c//4, query-head group g = c%4).
Each core computes its group's Q projection (256 cols of W_Q), the
group-shared K/V projections, 4 heads of attention over the full
sequence, and a partial output projection against the group's 256 rows
of W_O. The host sums the 4 group partials per batch (the "all-reduce")
and adds b_O.

On-core dataflow (all matmuls bf16 operands, fp32 PSUM accumulate):
  xbf  = bf16(x)                       DVE cast
  xT   = transpose(xbf)                PE transpose (bf16, 1 cyc/row)
  [K|V]^T = [wk|wv]^T x  per k-chunk   one packed matmul; K rows 0:64,
                                       V^T rows 64:128 of PSUM
  V    = transpose(V^T)                PE, 16 small transposes
  QT   = W_Q^T x  (q-dim on partitions), + b_Q on evac
  S^T  = KT_h^T @ QT_h   per head, [t, s] layout; head pairs sit on PE
         row-tiles (0,0)/(64,0)
  P^T  = exp(S^T / 8)    ScalarE, PSUM -> SBUF bf16
  ctx  = P^T_chunk^T @ [V | 1]   per-head contiguous half-chains (PSUM
         accumulation groups cannot interleave within a tile), pass A
         parked in SBUF fp32, pass B + DVE add in the next super-block
  ctx /= denom; transpose -> ctxT; out = ctxT^T @ W_O (partial, fp32)

The PE only reaches its 2.4 GHz p-state after ~3us of continuous
execution; an ACT-paced schedule leaves it flapping at 1.2 GHz. FILL
zero-stationary matmuls are appended to score accumulation groups
(exact +0) as p-state ballast to keep the PE above the Scalar engine's
pace and ramped.

b_V and b_O are applied on the host: b_V adds exactly
(tile(b_V) @ W_O_g) to every output row (softmax weights sum to 1).
"""

import numpy as np

S = 2048
DM = 1024
G = 4
H = 4  # heads per group
DK = 64
GQ = 256  # query width per group
B = 2
NK = DM // 128  # 8 contraction chunks
NT = S // 128  # 16 token chunks
SBLK = 512
NSB = S // SBLK  # 4 query super-blocks
PTS = 12  # P^T ring slots (token chunks in flight)
FILL = 0  # zero-filler matmuls per score group (p-state ballast)

_CACHED = {}


def _split_sync_waits(nc, drain_max=1, other_max=1):
    """This walrus build has a single sync-wait slot on CTRL-class
    instructions (Drain/NoOp); Tile's exit drain collects 3+. Move the
    excess onto preceding single-wait NOPs on the same engine."""
    import concourse.mybir as mybir
    import bass_rust

    n_split = 0
    for f in nc.m.functions:
        for bb in f.blocks:
            out = []
            changed = False
            for inst in bb.instructions:
                si = getattr(inst, "sync_info", None)
                limit = drain_max if type(inst).__name__ in ("InstDrain", "InstNoOp") else other_max
                if si is not None and len(si.on_wait) > limit:
                    waits = list(si.on_wait)
                    keep = waits[-limit:] if limit else []
                    head = waits[: len(waits) - limit]
                    for w in head:
                        out.append(
                            mybir.InstNoOp(
                                name=f"{inst.name}-wsp{n_split}",
                                engine=inst.engine,
                                sync_info=mybir.SyncInfo(on_wait=[w], on_update=[]),
                                bass_nofuse=True,
                            )
                        )
                        n_split += 1
                    inst.sync_info = bass_rust.SyncInfo(on_wait=keep, on_update=si.on_update)
                    changed = True
                out.append(inst)
            if changed:
                bb.instructions = out
    return n_split


def _build_nc(iters=1, fill=None, probe=0):
    import concourse.bass as bass
    import concourse.mybir as mybir
    import concourse.tile as tile
    from concourse import masks

    if fill is None:
        fill = FILL
    F32 = mybir.dt.float32
    BF = mybir.dt.bfloat16

    nc = bass.Bass("TRN2", target_bir_lowering=False, debug=False, num_devices=8)
    x = nc.dram_tensor("x", [S, DM], F32, kind="ExternalInput")
    wq = nc.dram_tensor("wq", [DM, GQ], F32, kind="ExternalInput")
    wk = nc.dram_tensor("wk", [DM, DK], F32, kind="ExternalInput")
    wv = nc.dram_tensor("wv", [DM, DK], F32, kind="ExternalInput")
    wo = nc.dram_tensor("wo", [GQ, DM], F32, kind="ExternalInput")
    bq = nc.dram_tensor("bq", [GQ], F32, kind="ExternalInput")
    bk = nc.dram_tensor("bk", [DK], F32, kind="ExternalInput")
    out = nc.dram_tensor("out", [S, DM], F32, kind="ExternalOutput")

    with tile.TileContext(nc) as tc:
        with (
            tc.tile_pool(name="const", bufs=1) as cpool,
            tc.tile_pool(name="wstg", bufs=3) as wstg,
            tc.tile_pool(name="wts", bufs=1) as wts,
            tc.tile_pool(name="xin", bufs=2) as xin,
            tc.tile_pool(name="acts", bufs=1) as acts,
            tc.tile_pool(name="outp", bufs=2) as outp,
            tc.tile_pool(name="ps_sc", bufs=4, space="PSUM") as ps_sc,
            tc.tile_pool(name="ps_ctx", bufs=2, space="PSUM") as ps_ctx,
        ):
            def _pipeline():
                # ---- constants ----
                ident_b = cpool.tile([128, 128], BF)
                masks.make_identity(nc, ident_b[:])
                # ident_lo[64+i, j] = delta(i, j): identity content on
                # partitions 64:128, for transposing stationaries that
                # live on the lower half.
                ident_lo = cpool.tile([128, 64], BF)
                nc.sync.dma_start(ident_lo[64:128, :], ident_b[:64, :64])
                ident_f = cpool.tile([128, 128], F32)
                masks.make_identity(nc, ident_f[:])
                bq_t = cpool.tile([128, 2], F32)
                for m in range(2):
                    nc.sync.dma_start(bq_t[:, m : m + 1], bq[m * 128 : (m + 1) * 128])
                bk_t = cpool.tile([64, 1], F32)
                nc.sync.dma_start(bk_t[:], bk[:])
                zeros_b = cpool.tile([128, 128], BF)
                nc.vector.memset(zeros_b[:], 0.0)
                ones1 = cpool.tile([1, 64], BF)
                nc.vector.memset(ones1[:], 1.0)

                # ---- weights: fp32 staging DMAs ride the Activation HWDGE
                # queue so the SP queue starts the x load immediately ----
                wq_bf = wts.tile([128, NK * GQ], BF)  # chunk k at cols [k*GQ,)
                kv_bf = wts.tile([128, NK * 128], BF)  # chunk k: [wk_k | wv_k]
                wo_bf = wts.tile([128, 2 * DM], BF)  # chunk cj at cols [cj*DM,)

                stg2 = wstg.tile([128, NK * GQ], F32, tag="stg")
                kv_view = stg2[:, : NK * 128].rearrange("p (k t q) -> p k t q", t=2, q=DK)
                nc.scalar.dma_start(
                    kv_view[:, :, 0, :], wk[:].rearrange("(k p) q -> p k q", p=128)
                )
                nc.scalar.dma_start(
                    kv_view[:, :, 1, :], wv[:].rearrange("(k p) q -> p k q", p=128)
                )
                nc.vector.tensor_copy(kv_bf[:], stg2[:, : NK * 128])

                stg = wstg.tile([128, NK * GQ], F32, tag="stg")
                nc.scalar.dma_start(
                    stg[:].rearrange("p (k q) -> p k q", q=GQ),
                    wq[:].rearrange("(k p) q -> p k q", p=128),
                )
                nc.vector.tensor_copy(wq_bf[:], stg[:])

                stg3 = wstg.tile([128, NK * GQ], F32, tag="stg")
                nc.scalar.dma_start(
                    stg3[:].rearrange("p (c n) -> p c n", n=DM),
                    wo[:].rearrange("(c p) n -> p c n", p=128),
                )
                nc.vector.tensor_copy(wo_bf[:], stg3[:])

                # ---- activations ----
                xT = acts.tile([128, NK * S], BF)  # xT[:, k*S + s]
                KT = acts.tile([128, S], BF)  # rows 64-127 duplicate 0-63
                VT = acts.tile([128, S], BF)  # V^T on rows 64:128
                Vb = acts.tile([128, NT * (DK + 1)], BF)  # [V | ones] per chunk
                QT = acts.tile([128, 2 * S], BF)  # m-tile m: heads 2m, 2m+1
                PT = acts.tile([128, PTS * 2 * SBLK], BF)  # ring, slot g2%PTS
                ctx_ab = [
                    acts.tile([128, 4 * GQ], BF, name=f"ctx{i}") for i in range(2)
                ]
                ctxT_ab = [
                    acts.tile([128, 2 * SBLK], BF, name=f"ctxT{i}") for i in range(2)
                ]
                ctxA = acts.tile([128, 2 * 260], F32)  # pass-A partials (2 heads)
                ctxs = acts.tile([128, 2 * 260], F32)  # A+B sums

                nc.vector.memset(
                    Vb[:].rearrange("p (t c) -> p t c", c=65)[:, :, DK], 1.0
                )

                xfs = {}

                def load_sg(sg):
                    xf = xin.tile([128, 4 * DM], F32, tag="xf")
                    nc.sync.dma_start(
                        xf[:].rearrange("p (c d) -> p c d", d=DM),
                        x[sg * SBLK : (sg + 1) * SBLK, :].rearrange(
                            "(c p) d -> p c d", p=128
                        ),
                    )
                    xfs[sg] = xf

                def sg_block(sg):
                    """cast + transpose + packed K/V proj + V chunks + KT dup
                    for one token super-block."""
                    if sg + 1 < NSB:
                        load_sg(sg + 1)
                    xf = xfs.pop(sg)
                    for k in range(NK):
                        ps = ps_sc.tile([128, 2 * SBLK], F32, tag="sc", name="pstr")
                        for i in range(4):
                            nc.tensor.transpose(
                                ps[:, i * 128 : (i + 1) * 128],
                                xf[:, i * DM + k * 128 : i * DM + (k + 1) * 128],
                                ident_f[:],
                            )
                        nc.vector.tensor_copy(
                            xT[:, k * S + sg * SBLK : k * S + (sg + 1) * SBLK],
                            ps[:, :SBLK],
                        )
                    ps2 = ps_sc.tile([128, 2 * SBLK], F32, tag="sc", name="pskv")
                    for k in range(NK):
                        nc.tensor.matmul(
                            ps2[:, :SBLK],
                            kv_bf[:, k * 128 : (k + 1) * 128],
                            xT[:, k * S + sg * SBLK : k * S + (sg + 1) * SBLK],
                            start=(k == 0),
                            stop=(k == NK - 1),
                        )
                    nc.vector.tensor_scalar_add(
                        KT[:64, sg * SBLK : (sg + 1) * SBLK], ps2[:64, :SBLK], bk_t[:]
                    )
                    nc.vector.tensor_copy(
                        VT[64:128, sg * SBLK : (sg + 1) * SBLK], ps2[64:128, :SBLK]
                    )
                    nc.sync.dma_start(
                        KT[64:128, sg * SBLK : (sg + 1) * SBLK],
                        KT[:64, sg * SBLK : (sg + 1) * SBLK],
                    )
                    for t in range(sg * 4, sg * 4 + 4):
                        vps = ps_sc.tile([128, 2 * SBLK], BF, tag="sc", name="vtr")
                        nc.tensor.transpose(
                            vps[:, :DK],
                            VT[64:128, t * 128 : (t + 1) * 128],
                            ident_lo[64:128, :],
                        )
                        nc.vector.tensor_copy(Vb[:, t * 65 : t * 65 + DK], vps[:, :DK])

                def q_block(m, sb):
                    ps = ps_sc.tile([128, 2 * SBLK], F32, tag="sc", name="psq")
                    for k in range(NK):
                        nc.tensor.matmul(
                            ps[:, :SBLK],
                            wq_bf[:, k * GQ + m * 128 : k * GQ + (m + 1) * 128],
                            xT[:, k * S + sb * SBLK : k * S + (sb + 1) * SBLK],
                            start=(k == 0),
                            stop=(k == NK - 1),
                        )
                    nc.vector.tensor_scalar_add(
                        QT[:, m * S + sb * SBLK : m * S + (sb + 1) * SBLK],
                        ps[:, :SBLK],
                        bq_t[:, m : m + 1],
                    )

                def g2slot(sb, p, t):
                    return (((sb * 2 + p) * NT + t) % PTS) * 2 * SBLK

                def scores_t(sb, p, t):
                    """score matmuls + exp for one token chunk of head pair
                    p (heads 2p, 2p+1) into the P^T ring."""
                    slot = g2slot(sb, p, t)
                    sc = ps_sc.tile([128, 2 * SBLK], F32, tag="sc", name="scs")
                    for hl in range(2):
                        nc.tensor.matmul(
                            sc[:, hl * SBLK : (hl + 1) * SBLK],
                            KT[hl * 64 : (hl + 1) * 64, t * 128 : (t + 1) * 128],
                            QT[hl * 64 : (hl + 1) * 64,
                               p * S + sb * SBLK : p * S + (sb + 1) * SBLK],
                        )
                    nc.scalar.activation(
                        PT[:, slot : slot + 2 * SBLK],
                        sc[:],
                        mybir.ActivationFunctionType.Exp,
                        scale=0.125,
                    )

                # Context computed TRANSPOSED with the [V | 1] chunk as the
                # stationary operand: one N=512 matmul per (head, token
                # chunk) accumulating ctxT_h = [V|1]^T P_h^T = [ctx^T; denom]
                # in a per-head PSUM tile (rows 0:65). Small-N matmuls pay a
                # ~146 ns access-latency floor, so wide-N beats the natural
                # [s,65]-output form by ~3x. Normalization broadcasts 1/denom
                # over 64 partitions with a rank-1 ones matmul, then one DVE
                # multiply per head; odd heads are DMA-relocated to the upper
                # partition half of their pair's stationary tile.
                # Natural-form context: ctx[s, dk+1] accumulated per
                # (head, query-block) in 16-deep chains of the [V|1] moving
                # operand. PSUM groups must stay contiguous per tile, so each
                # head runs as two sequential half-chain bursts (t 0-7, 8-15);
                # pass A parks in SBUF fp32, pass B + a DVE add combine in the
                # next stream.
                cps_by = {}

                def ctx_burst(sb, p, hl, t0, t1):
                    if probe:
                        return
                    if (sb, p) not in cps_by:
                        cps_by[(sb, p)] = {}
                    cps = ps_sc.tile([128, 2 * SBLK], F32, tag="sc", name=f"cps{hl}")
                    cps_by[(sb, p)][(hl, t0)] = cps
                    for sc_i in range(4):
                        for t in range(t0, t1 + 1):
                            slot = g2slot(sb, p, t)
                            nc.tensor.matmul(
                                cps[:, sc_i * 65 : sc_i * 65 + 65],
                                PT[:, slot + hl * SBLK + sc_i * 128 : slot + hl * SBLK + (sc_i + 1) * 128],
                                Vb[:, t * 65 : (t + 1) * 65],
                                start=(t == t0),
                                stop=(t == t1),
                            )
                    if t0 == 0:  # pass A: park partials in SBUF
                        nc.vector.tensor_copy(
                            ctxA[:, hl * 260 : (hl + 1) * 260], cps[:, : 4 * 65]
                        )

                def ctx_addevac(sb, p):
                    if probe:
                        cps_by.pop((sb, p), None)
                        return
                    ctx_sb = ctx_ab[sb % 2]
                    cps = cps_by.pop((sb, p))
                    for hl in range(2):
                        h = 2 * p + hl
                        nc.vector.tensor_add(
                            ctxs[:, hl * 260 : (hl + 1) * 260],
                            cps[(hl, 8)][:, : 4 * 65],
                            ctxA[:, hl * 260 : (hl + 1) * 260],
                        )
                        rc = xin.tile([128, 4], F32, tag="rc")
                        nc.vector.reciprocal(
                            rc[:],
                            ctxs[:, hl * 260 : (hl + 1) * 260].rearrange(
                                "p (sc c) -> p sc c", c=65
                            )[:, :, DK],
                        )
                        for sc_i in range(4):
                            nc.vector.tensor_scalar_mul(
                                ctx_sb[:, sc_i * GQ + h * DK : sc_i * GQ + (h + 1) * DK],
                                ctxs[:, hl * 260 + sc_i * 65 : hl * 260 + sc_i * 65 + DK],
                                rc[:, sc_i : sc_i + 1],
                            )

                def ctxT_block(sb):
                    if probe:
                        return
                    ctx_sb, ctxT_sb = ctx_ab[sb % 2], ctxT_ab[sb % 2]
                    for cj in range(2):
                        ps = ps_sc.tile([128, 2 * SBLK], BF, tag="sc", name="pst")
                        for sc_i in range(4):
                            nc.tensor.transpose(
                                ps[:, sc_i * 128 : (sc_i + 1) * 128],
                                ctx_sb[:, sc_i * GQ + cj * 128 : sc_i * GQ + (cj + 1) * 128],
                                ident_b[:],
                            )
                        nc.vector.tensor_copy(
                            ctxT_sb[:, cj * SBLK : (cj + 1) * SBLK], ps[:, :SBLK]
                        )

                def out_half(sb, half):
                    ctxT_sb = ctxT_ab[sb % 2]
                    ot = outp.tile([128, 2 * DM], F32, tag="ot")
                    if probe:
                        nc.vector.tensor_copy(ot[:, :SBLK], PT[:, :SBLK])
                        row = sb * SBLK + half * 256
                        nc.sync.dma_start(
                            out[row : row + 256, :].rearrange("(c p) d -> p c d", p=128),
                            ot[:].rearrange("p (c d) -> p c d", d=DM),
                        )
                        return
                    for ci in range(2):
                        sc_i = half * 2 + ci
                        ps = ps_sc.tile([128, 2 * SBLK], F32, tag="sc", name="pso")
                        for nb in range(2):
                            for cj in range(2):
                                nc.tensor.matmul(
                                    ps[:, nb * SBLK : (nb + 1) * SBLK],
                                    ctxT_sb[:, cj * SBLK + sc_i * 128 : cj * SBLK + (sc_i + 1) * 128],
                                    wo_bf[:, cj * DM + nb * SBLK : cj * DM + (nb + 1) * SBLK],
                                    start=(cj == 0),
                                    stop=(cj == 1),
                                )
                        nc.vector.tensor_copy(ot[:, ci * DM : (ci + 1) * DM], ps[:, :DM])
                    row = sb * SBLK + half * 256
                    nc.sync.dma_start(
                        out[row : row + 256, :].rearrange("(c p) d -> p c d", p=128),
                        ot[:].rearrange("p (c d) -> p c d", d=DM),
                    )

                def stream_sp(sb, p, prework):
                    """One head-pair pass: 16 score/exp tiles; pass-A ctx
                    bursts run at t=8/10 once their P^T chunks exist;
                    prework[t] fills the ACT-paced slack."""
                    for t in range(NT):
                        scores_t(sb, p, t)
                        if t == 8 or t == 10:
                            ctx_burst(sb, p, (t - 8) // 2, 0, 7)
                        for fn in prework.get(t, ()):
                            fn()

                # ---- software-pipelined schedule: 8 passes ----
                load_sg(0)
                sg_block(0)
                q_block(0, 0)
                q_block(1, 0)
                stream_sp(
                    0, 0,
                    {
                        1: [lambda: sg_block(1)],
                        5: [lambda: sg_block(2)],
                        9: [lambda: sg_block(3)],
                    },
                )
                for sb in range(NSB):
                    for p in range(2):
                        if sb == 0 and p == 0:
                            continue
                        if p == 1:
                            pv, pp = sb, 0  # previous pass: same sb, pass 0
                        else:
                            pv, pp = sb - 1, 1
                        prework = {
                            0: [lambda: ctx_burst(pv, pp, 0, 8, NT - 1)],
                            2: [lambda: ctx_burst(pv, pp, 1, 8, NT - 1)],
                            4: [lambda: ctx_addevac(pv, pp)],
                        }
                        if p == 0:
                            # sb-1 fully evacuated now: transpose + project
                            prework[6] = [lambda: ctxT_block(sb - 1)]
                            prework[9] = [lambda: out_half(sb - 1, 0)]
                            prework[12] = [lambda: out_half(sb - 1, 1)]
                            prework[14] = [lambda: q_block(1, sb)]
                        elif sb + 1 < NSB:
                            prework[14] = [lambda: q_block(0, sb + 1)]
                        stream_sp(sb, p, prework)

                last = NSB - 1
                ctx_burst(last, 1, 0, 8, NT - 1)
                ctx_burst(last, 1, 1, 8, NT - 1)
                ctx_addevac(last, 1)
                ctxT_block(last)
                out_half(last, 0)
                out_half(last, 1)

            if iters == 1:
                _pipeline()
            else:
                with tc.For_i(0, iters):
                    _pipeline()

    _split_sync_waits(nc)
    return nc


def kernel(x, W_Q, b_Q, W_K, b_K, W_V, b_V, W_O, b_O):
    from concourse.bass_utils import run_bass_kernel_spmd

    x = np.asarray(x, np.float32)
    W_Q, b_Q = np.asarray(W_Q, np.float32), np.asarray(b_Q, np.float32)
    W_K, b_K = np.asarray(W_K, np.float32), np.asarray(b_K, np.float32)
    W_V, b_V = np.asarray(W_V, np.float32), np.asarray(b_V, np.float32)
    W_O, b_O = np.asarray(W_O, np.float32), np.asarray(b_O, np.float32)

    if "nc" not in _CACHED:
        _CACHED["nc"] = _build_nc()
    nc = _CACHED["nc"]

    in_maps = []
    for c in range(8):
        b, g = divmod(c, 4)
        in_maps.append(
            {
                "x": np.ascontiguousarray(x[b]),
                "wq": np.ascontiguousarray(W_Q[:, g * GQ : (g + 1) * GQ]),
                "wk": np.ascontiguousarray(W_K[g]),
                "wv": np.ascontiguousarray(W_V[g]),
                "wo": np.ascontiguousarray(W_O[g * GQ : (g + 1) * GQ, :]),
                "bq": np.ascontiguousarray(b_Q[g * GQ : (g + 1) * GQ]),
                "bk": np.ascontiguousarray(b_K[g]),
            }
        )
    res = run_bass_kernel_spmd(nc, in_maps, list(range(8)))

    out = np.zeros((B, S, DM), np.float32)
    for c in range(8):
        b, g = divmod(c, 4)
        out[b] += res.results[c]["out"]
    # host-side bias terms: b_O, plus b_V's exact contribution
    # (softmax rows sum to 1 -> ctx bias = tile(b_V[g]) per head)
    bv_full = np.concatenate([np.tile(b_V[g], H) for g in range(G)])  # [1024]
    out += (b_O + bv_full @ W_O)[None, None, :]
    return out
